# revision 21
# baseline (speedup 1.0000x reference)
"""KAN-SE (squeeze-excite with 2-layer KAN MLP) Trainium2 kernel.

Full-input contract: kernel(**inputs) takes the complete (32, 512, 64, 64)
f32 batch plus KAN weights, shards the batch across 8 NeuronCores (4
samples per core, data-parallel, weights replicated), and returns the
full f32 output.

The device pipeline runs entirely in bf16 for the bulk data (the gate is
computed in f32): x is rounded to bf16 on the HOST and the device input/
output tensors are declared bf16, which HALVES HBM traffic in both
directions (16.8 MiB in + 16.8 MiB out per core instead of 33.5+33.5).
The result is upcast to f32 on the host. bf16 rounding of x costs ~0.2%
relative error - an order of magnitude inside the tolerance - and removes
any need for on-device dtype conversion, staging pools, or casting DMAs.

Per-core device program (pure SPMD, no collectives):
  - ALL 16 tiles (4 samples x 4 channel-groups of (128, 4096) bf16) load
    up front on the SP HWDGE ring straight into the resident pool (128
    KiB/partition total - the whole batch stays on-chip); the loads carry
    no waits at all.
  - Per-channel raw sums via ScalarE in-place Copy activations whose
    accum_out produces the sum; sum blocks are emitted one sample ahead
    of the KAN so the in-order ScalarE stream never stalls them behind a
    sigmoid that waits on the PE.
  - Per-sample 2-layer KAN on the sums (f32 math on tiny tensors, bf16
    weights/bases feeding the PE). Mean normalization is folded into the
    layer-1 tables / base weights; activations via Sigmoid only (silu(x)
    = x*sigmoid(x) with a DVE multiply). Layer-1 B-spline bases for all
    4 channel groups are computed by ONE batched Cox-de-Boor chain (~20
    VectorE ops) using stride-0 broadcast APs. The einsums run as paired
    PE matmuls: layer 1 packs the 36 (128x64) blocks pairwise into 18
    (128x128) stationary loads with a 2-column rhs (one DVE add fixes up
    h[o] = ps[o,0] + ps[64+o,1]); layer 2 stacks spline blocks k=2j,2j+1
    across partitions (contraction 64 -> 128), 5 matmuls per out-group.
  - Gate scales applied in-place on the bf16 tiles (VectorE, f32 gate
    scalar per partition), stores on the SP ring FIFO behind the loads.
"""

import numpy as np

# ---- problem constants (hardcoded per contract; do not read spec/reference) ----
B, C, H, W = 32, 512, 64, 64
HIDDEN = 64            # max(16, 512 // 8)
KB = 8                 # GRID_SIZE + SPLINE_ORDER = 5 + 3
NCORES = 8
NS = B // NCORES       # samples per core = 4
NG = C // 128          # channel groups of 128 = 4
HWPIX = H * W          # 4096
NB1 = NG * KB + NG     # layer-1 weight blocks: 32 spline + 4 base = 36
NP1 = NB1 // 2         # 18 paired stationary loads

# Batched layer-1 grid table column layout (widths per Cox-de-Boor level):
# [G0: 4 groups x 12 | A1: 4x10 | C1: 4x10 | A2: 4x9 | C2: 4x9 | A3: 4x8 | C3: 4x8]
_B_OFF = {"G0": 0, (1, "A"): 48, (1, "C"): 88, (2, "A"): 128, (2, "C"): 164,
          (3, "A"): 200, (3, "C"): 232}
_B_W = 264

# layer-2 (per-group) gtab layout: [G0(12) | -g_i k=1(10) | g_{i+2} k=1(10)
#                      | -g_i k=2(9) | g_{i+3} k=2(9) | -g_i k=3(8) | g_{i+4} k=3(8)]
_GT_OFF = {"G0": 0, 1: (12, 22), 2: (32, 41), 3: (50, 58)}
_GT_W = 66


def _grid_tables(grid_row: np.ndarray, pre_scale: float = 1.0):
    """(128, 66) constant table + per-level reciprocals for the per-group
    Cox-de-Boor chain (used for layer 2)."""
    g = np.asarray(grid_row, np.float64) * pre_scale
    assert g.shape == (12,)
    h = g[1] - g[0]
    tab = np.zeros((_GT_W,), np.float64)
    tab[0:12] = g
    rs = {}
    for k in (1, 2, 3):
        w = 11 - k
        aoff, coff = _GT_OFF[k]
        tab[aoff:aoff + w] = -g[:w]          # -g_i,      i = 0..10-k
        tab[coff:coff + w] = g[k + 1:12]     # g_{i+k+1}, i = 0..10-k
        rs[k] = float(np.float32(1.0 / (k * h)))
    full = np.tile(tab.astype(np.float32)[None, :], (128, 1))
    return np.ascontiguousarray(full), rs


def _grid_tables_batched(grid_row: np.ndarray, pre_scale: float):
    """(128, 264) table for the group-batched layer-1 chain, with the
    per-level reciprocals folded in:
      A'_k[i] = -g_i / (k h)   (chain adds s/(k h))
      C'_k[i] = g_{i+k+1} / (k h)  (chain subtracts s/(k h))."""
    g = np.asarray(grid_row, np.float64) * pre_scale
    h = g[1] - g[0]
    tab = np.zeros((_B_W,), np.float64)
    tab[0:48] = np.tile(g, 4)
    rs = {}
    for k in (1, 2, 3):
        w = 11 - k
        r = 1.0 / (k * h)
        rs[k] = float(np.float32(r))
        tab[_B_OFF[(k, "A")]:_B_OFF[(k, "A")] + 4 * w] = np.tile(-g[:w] * r, 4)
        tab[_B_OFF[(k, "C")]:_B_OFF[(k, "C")] + 4 * w] = np.tile(g[k + 1:12] * r, 4)
    full = np.tile(tab.astype(np.float32)[None, :], (128, 1))
    return np.ascontiguousarray(full), rs


def _host_prep(inputs):
    """Rearrange weights into the SBUF layouts the device program uses."""
    import ml_dtypes
    f32 = np.float32
    bf16 = ml_dtypes.bfloat16
    base_w1 = np.asarray(inputs["base_w1"], f32)      # (64, 512)
    spline_w1 = np.asarray(inputs["spline_w1"], f32)  # (64, 512, 8)
    scaler1 = np.asarray(inputs["scaler1"], f32)      # (64, 512)
    base_w2 = np.asarray(inputs["base_w2"], f32)      # (512, 64)
    spline_w2 = np.asarray(inputs["spline_w2"], f32)  # (512, 64, 8)
    scaler2 = np.asarray(inputs["scaler2"], f32)      # (512, 64)

    # Layer-1 blocks in rhs-column order: j<32 spline (g = j//8, k = j%8),
    # j>=32 base path for group g = j-32 (consumes q = s*sigmoid(s/HWPIX) =
    # HWPIX*silu(mean), so the base weights carry the 1/HWPIX).
    sw1 = (spline_w1 * scaler1[:, :, None]).reshape(HIDDEN, NG, 128, KB)
    sw1 = sw1.transpose(2, 1, 3, 0).reshape(128, NG * KB * HIDDEN)
    w1t = base_w1.reshape(HIDDEN, NG, 128).transpose(2, 1, 0).reshape(128, NG * HIDDEN)
    w1t = w1t / float(HWPIX)
    wpack1 = np.concatenate([sw1, w1t], axis=1)       # (128, 36*64)

    # Layer 2: base blocks w2t[p, o] = base_w2[o, p]; spline blocks stacked
    # pairwise across partitions (contraction 64 -> 128).
    w2t = np.ascontiguousarray(base_w2.T)             # (64, 512)
    sw2 = (spline_w2 * scaler2[:, :, None]).transpose(1, 2, 0)  # (64, 8, 512)
    w2pack = np.zeros((128, NG * 4 * 128), f32)
    for og in range(NG):
        for j in range(4):
            blk = slice((og * 4 + j) * 128, (og * 4 + j + 1) * 128)
            w2pack[0:64, blk] = sw2[:, 2 * j, 128 * og:128 * (og + 1)]
            w2pack[64:128, blk] = sw2[:, 2 * j + 1, 128 * og:128 * (og + 1)]

    # Layer 1 evaluates splines on raw per-channel sums: fold the 1/HWPIX
    # mean into the knot tables.
    gtb1, rs1 = _grid_tables_batched(np.asarray(inputs["grid1"], f32)[0],
                                     pre_scale=float(HWPIX))
    gt2, rs2 = _grid_tables(np.asarray(inputs["grid2"], f32)[0])

    tensors = {
        "wpack1": np.ascontiguousarray(wpack1.astype(bf16)),
        "w2pack": np.ascontiguousarray(w2pack.astype(bf16)),
        "w2t": np.ascontiguousarray(w2t.astype(bf16)),
        "gtb1": gtb1,
        "gt2": gt2,
    }
    return tensors, rs1, rs2


def _emit_bsplines(nc, mybir, pool, gt_sb, x_ap, out_ap, p, rs):
    """Per-group cubic B-spline bases (layer 2): x (one value per partition)
    -> out_ap (p, 8). Cox-de-Boor on VectorE."""
    f32 = mybir.dt.float32
    Alu = mybir.AluOpType
    ge = pool.tile([128, 12], f32, tag="ge", bufs=4)
    nc.vector.tensor_scalar(
        out=ge[:p], in0=gt_sb[:p, 0:12], scalar1=x_ap, scalar2=None, op0=Alu.is_le
    )
    bprev = pool.tile([128, 11], f32, tag="b0", bufs=4)
    nc.vector.tensor_tensor(bprev[:p], ge[:p, 0:11], ge[:p, 1:12], Alu.subtract)
    for k in (1, 2, 3):
        w = 11 - k
        aoff, coff = _GT_OFF[k]
        a_t = pool.tile([128, 10], f32, tag="bsA", bufs=4)
        c_t = pool.tile([128, 10], f32, tag="bsC", bufs=4)
        nc.vector.tensor_scalar(
            out=a_t[:p, :w], in0=gt_sb[:p, aoff:aoff + w], scalar1=x_ap,
            scalar2=rs[k], op0=Alu.add, op1=Alu.mult,
        )
        nc.vector.tensor_scalar(
            out=c_t[:p, :w], in0=gt_sb[:p, coff:coff + w], scalar1=x_ap,
            scalar2=rs[k], op0=Alu.subtract, op1=Alu.mult,
        )
        if k < 3:
            bnext = pool.tile([128, 10], f32, tag="bn", bufs=4)
            outp = bnext[:p, :w]
        else:
            outp = out_ap
        nc.vector.tensor_tensor(c_t[:p, :w], c_t[:p, :w], bprev[:p, 1:w + 1], Alu.mult)
        nc.vector.tensor_tensor(outp, a_t[:p, :w], bprev[:p, 0:w], Alu.mult)
        nc.vector.tensor_tensor(outp, outp, c_t[:p, :w], Alu.add)
        if k < 3:
            bprev = bnext


def _emit_bsplines_batched(nc, mybir, pool, spool, gtb_sb, sT, out_ap, rs):
    """Group-batched layer-1 Cox-de-Boor: sums sT (128, 4) -> out_ap
    viewed (128, 4, 8). One VectorE op per step for all 4 groups, using
    stride-0 broadcast APs of sT / (sT * rs_k) along the basis index."""
    f32 = mybir.dt.float32
    Alu = mybir.AluOpType

    # sr[:, 4(k-1):4k] = sT * rs_k  (the s/(k h) broadcast operand)
    sr = spool.tile([128, 12], f32, tag="sr")
    for k in (1, 2, 3):
        nc.vector.tensor_scalar(
            out=sr[:, 4 * (k - 1):4 * k], in0=sT[:], scalar1=rs[k], scalar2=None,
            op0=Alu.mult,
        )

    g3 = lambda ap, w: ap.rearrange("p (g i) -> p g i", g=NG)  # noqa: E731
    ge = pool.tile([128, 48], f32, tag="ge4")
    gev = g3(ge[:], 12)
    nc.vector.tensor_tensor(
        gev, g3(gtb_sb[:, 0:48], 12), sT[:].broadcast_to([128, NG, 12]), Alu.is_le
    )
    bprev = pool.tile([128, 44], f32, tag="b04")
    bpv = g3(bprev[:], 11)
    nc.vector.tensor_tensor(bpv, gev[:, :, 0:11], gev[:, :, 1:12], Alu.subtract)
    for k in (1, 2, 3):
        w = 11 - k
        srb = sr[:, 4 * (k - 1):4 * k].broadcast_to([128, NG, w])
        av = g3(gtb_sb[:, _B_OFF[(k, "A")]:_B_OFF[(k, "A")] + 4 * w], w)
        cv = g3(gtb_sb[:, _B_OFF[(k, "C")]:_B_OFF[(k, "C")] + 4 * w], w)
        a_t = pool.tile([128, 40], f32, tag="bsA4", bufs=2)
        c_t = pool.tile([128, 40], f32, tag="bsC4", bufs=2)
        atv = g3(a_t[:, :4 * w], w)
        ctv = g3(c_t[:, :4 * w], w)
        # A = (s - g_i)/(kh) = s*r + (-g_i*r);  C = (g_{i+k+1} - s)/(kh)
        nc.vector.tensor_tensor(atv, av, srb, Alu.add)
        nc.vector.tensor_tensor(ctv, cv, srb, Alu.subtract)
        if k < 3:
            bnext = pool.tile([128, 40], f32, tag="bn4", bufs=2)
            outp = g3(bnext[:, :4 * w], w)
        else:
            outp = out_ap
        nc.vector.tensor_tensor(ctv, ctv, bpv[:, :, 1:w + 1], Alu.mult)
        nc.vector.tensor_tensor(outp, atv, bpv[:, :, 0:w], Alu.mult)
        nc.vector.tensor_tensor(outp, outp, ctv, Alu.add)
        if k < 3:
            bprev, bpv = bnext, outp


def _build_nc(rs1, rs2):
    import concourse.bacc as bacc
    import concourse.bass as bass  # noqa: F401
    import concourse.mybir as mybir
    from concourse.tile import TileContext

    f32 = mybir.dt.float32
    bf16 = mybir.dt.bfloat16
    Alu = mybir.AluOpType
    Act = mybir.ActivationFunctionType

    # Bacc (not plain Bass): its compile() runs move_matmul_waits_to_ldweights
    # + generate_event_semaphores, which split multi-waits down to the 1-wait-
    # per-instruction TRN2 ISA limit that walrus enforces.
    nc = bacc.Bacc("TRN2", target_bir_lowering=False)
    x_d = nc.declare_dram_parameter("x", [NS, C, H, W], bf16, isOutput=False)
    wp1_d = nc.declare_dram_parameter("wpack1", [128, NB1 * HIDDEN], bf16, isOutput=False)
    wp2_d = nc.declare_dram_parameter("w2pack", [128, NG * 4 * 128], bf16, isOutput=False)
    w2t_d = nc.declare_dram_parameter("w2t", [HIDDEN, C], bf16, isOutput=False)
    gtb1_d = nc.declare_dram_parameter("gtb1", [128, _B_W], f32, isOutput=False)
    gt2_d = nc.declare_dram_parameter("gt2", [128, _GT_W], f32, isOutput=False)
    y_d = nc.declare_dram_parameter("y", [NS, C, H, W], bf16, isOutput=True)

    with TileContext(nc) as tc:
        with (
            tc.tile_pool(name="consts", bufs=1) as cpool,
            tc.tile_pool(name="xdata", bufs=NS * NG // 2) as xpool,
            tc.tile_pool(name="small", bufs=3) as spool,
            tc.tile_pool(name="bspl", bufs=1) as bpool,
            tc.tile_pool(name="psum", bufs=2, space="PSUM") as ppool,
        ):
            # Constants on the ACT HWDGE ring: off the x-load (SP) ring.
            # Grid tables first: the first sample's B-splines need them early.
            gtb1_sb = cpool.tile([128, _B_W], f32)
            nc.scalar.dma_start(gtb1_sb[:], gtb1_d[:, :])
            gt2_sb = cpool.tile([128, _GT_W], f32)
            nc.scalar.dma_start(gt2_sb[:], gt2_d[:, :])
            wp1_sb = cpool.tile([128, NB1 * HIDDEN], bf16)
            nc.scalar.dma_start(wp1_sb[:], wp1_d[:, :])
            wp2_sb = cpool.tile([128, NG * 4 * 128], bf16)
            nc.scalar.dma_start(wp2_sb[:], wp2_d[:, :])
            w2t_sb = cpool.tile([HIDDEN, C], bf16)
            nc.scalar.dma_start(w2t_sb[:], w2t_d[:, :])

            # Pre-touch every const tile on VectorE: the DMA-completion wait
            # lands on these throwaway copies, so later DVE consumers (notably
            # TensorScalarPtr ops, whose ISA format has a single wait slot)
            # never need a DMA wait of their own.
            touch = cpool.tile([128, 8], f32)
            for i, ct in enumerate((wp1_sb, wp2_sb, gtb1_sb, gt2_sb)):
                nc.vector.tensor_copy(touch[:, i:i + 1], ct[:, 0:1])
            nc.vector.tensor_copy(touch[:HIDDEN, 4:5], w2t_sb[:, 0:1])
            # Same for TensorE: the LDWEIGHTS sub-instruction also has a single
            # wait slot, so absorb each weight tile's DMA wait into a throwaway
            # 1-column matmul before the real accumulation chains.
            pt_ps = ppool.tile([1, 4], f32, tag="pt")
            for i, ct in enumerate((wp1_sb, wp2_sb)):
                nc.tensor.matmul(pt_ps[0:1, i:i + 1], ct[:, 0:1], ct[:, 0:1],
                                 start=True, stop=True)
            nc.tensor.matmul(pt_ps[0:1, 2:3], w2t_sb[:HIDDEN, 0:1],
                             w2t_sb[:HIDDEN, 0:1], start=True, stop=True)

            # ---- ALL x loads up front on the SP HWDGE ring (bf16, no
            # staging, no waits: the whole batch is SBUF-resident).
            # 2 MiB pair-DMAs (channel groups 2q, 2q+1 per transfer) for
            # better per-DMA efficiency; group tiles are views of the pair.
            xps = [[None] * (NG // 2) for _ in range(NS)]
            xts = [[None] * NG for _ in range(NS)]
            for n in range(NS):
                for q in range(NG // 2):
                    xp = xpool.tile([128, 2 * HWPIX], bf16, tag="xt")
                    src = x_d[n, 256 * q:256 * (q + 1)].rearrange(
                        "(g p) h w -> p g (h w)", g=2
                    )
                    nc.sync.dma_start(
                        xp[:].rearrange("p (g x) -> p g x", g=2), src
                    )
                    xps[n][q] = xp
                    xts[n][2 * q] = xp[:, 0:HWPIX]
                    xts[n][2 * q + 1] = xp[:, HWPIX:2 * HWPIX]

            sTs = [None] * NS

            def emit_sums(n):
                # per-channel raw sums, split across engines: groups 0,1 via
                # ScalarE in-place Copy+accum_out; groups 2,3 via VectorE
                # tensor_tensor_reduce (in-place max(x,x) identity whose
                # accumulator is the free-dim sum, at 2x bf16 TT rate)
                sT_n = spool.tile([128, NG], f32, tag="sT", bufs=4)
                sTs[n] = sT_n
                for g in (0, 1):
                    nc.scalar.activation(
                        xts[n][g], xts[n][g], Act.Copy,
                        accum_out=sT_n[:, g:g + 1],
                    )
                for g in (2, 3):
                    nc.vector.tensor_scalar(
                        out=xts[n][g], in0=xts[n][g], scalar1=1.0, scalar2=1.0,
                        op0=Alu.mult, op1=Alu.mult, accum_out=sT_n[:, g:g + 1],
                    )

            # Sum blocks one sample AHEAD of the KAN: on the in-order ScalarE
            # stream sample n+1's sums run before sample n's sigmoid chain
            # (which waits on the PE), keeping the gate cadence at the
            # engine-throughput limit rather than the serial-chain limit.
            emit_sums(0)
            for n in range(NS):
                if n + 1 < NS:
                    emit_sums(n + 1)
                sT = sTs[n]

                # ---- KAN layer 1: 18 paired matmuls; bfq holds the 36
                # rhs betas (32 spline via batched chain + 4 base) ----
                bfq = spool.tile([128, NB1], bf16, tag="bfq")
                bff = spool.tile([128, NG * KB], f32, tag="bff")
                sg = spool.tile([128, NG], f32, tag="sg")
                nc.scalar.activation(sg[:], sT[:], Act.Sigmoid, scale=1.0 / HWPIX)
                # base-path betas: q = s * sigmoid(s/HWPIX)  (cols 32..35)
                nc.vector.tensor_tensor(bfq[:, NG * KB:NB1], sg[:], sT[:], Alu.mult)
                _emit_bsplines_batched(
                    nc, mybir, bpool, spool, gtb1_sb, sT,
                    bff[:].rearrange("p (g i) -> p g i", g=NG), rs1,
                )
                nc.vector.tensor_copy(bfq[:, 0:NG * KB], bff[:])
                psA = ppool.tile([128, 2], f32, tag="ps1")
                for j in range(NP1):
                    nc.tensor.matmul(
                        psA[:, 0:2], wp1_sb[:, 128 * j:128 * (j + 1)],
                        bfq[:, 2 * j:2 * j + 2],
                        start=(j == 0), stop=(j == NP1 - 1),
                    )
                hv = spool.tile([HIDDEN, 1], f32, tag="hv")
                # DVE may read only one PSUM operand per instruction
                nc.vector.tensor_copy(hv[:], psA[HIDDEN:128, 1:2])
                nc.vector.tensor_tensor(hv[:], hv[:], psA[0:HIDDEN, 0:1], Alu.add)

                # ---- inter-layer SiLU (t1 = h*sigmoid(h)), layer 2 ----
                sg1 = spool.tile([HIDDEN, 1], f32, tag="sg1")
                nc.scalar.activation(sg1[:], hv[:], Act.Sigmoid)
                t1 = spool.tile([HIDDEN, 1], f32, tag="t1")
                nc.vector.tensor_tensor(t1[:], sg1[:], hv[:], Alu.mult)
                sg2 = spool.tile([HIDDEN, 1], f32, tag="sg2")
                nc.scalar.activation(sg2[:], t1[:], Act.Sigmoid)
                silu2 = spool.tile([HIDDEN, 1], bf16, tag="silu2")
                nc.vector.tensor_tensor(silu2[:], sg2[:], t1[:], Alu.mult)
                b2f = spool.tile([HIDDEN, KB], f32, tag="b2f")
                _emit_bsplines(nc, mybir, bpool, gt2_sb, t1[:, 0:1], b2f[:], HIDDEN, rs2)

                # stacked rhs: stk[0:64, j] = b2f[:, 2j], stk[64:128, j] = b2f[:, 2j+1]
                stk = spool.tile([128, 4], bf16, tag="stk")
                nc.vector.tensor_copy(stk[0:HIDDEN, 0:4], b2f[:, 0:KB:2])
                nc.vector.tensor_copy(stk[HIDDEN:128, 0:4], b2f[:, 1:KB:2])

                ps2 = ppool.tile([128, NG], f32, tag="ps2")
                for og in range(NG):
                    nc.tensor.matmul(
                        ps2[:, og:og + 1], w2t_sb[:, 128 * og:128 * (og + 1)],
                        silu2[:, 0:1], start=True, stop=False,
                    )
                    for j in range(4):
                        blk = slice((og * 4 + j) * 128, (og * 4 + j + 1) * 128)
                        nc.tensor.matmul(
                            ps2[:, og:og + 1], wp2_sb[:, blk], stk[:, j:j + 1],
                            start=False, stop=(j == 3),
                        )

                gate = spool.tile([128, NG], f32, tag="gate")
                nc.scalar.activation(gate[:], ps2[:], Act.Sigmoid)

                # ---- scale resident bf16 tiles in place (groups 0,1 on
                # ScalarE, 2,3 on VectorE), then store 2 MiB pairs on the
                # ACT HWDGE ring, overlapping the SP-ring loads ----
                for g in (0, 1):
                    nc.scalar.mul(xts[n][g], xts[n][g], gate[:, g:g + 1])
                for g in (2, 3):
                    nc.vector.tensor_scalar(
                        out=xts[n][g], in0=xts[n][g],
                        scalar1=gate[:, g:g + 1], scalar2=None, op0=Alu.mult,
                    )
                for q in range(NG // 2):
                    dst = y_d[n, 256 * q:256 * (q + 1)].rearrange(
                        "(g p) h w -> p g (h w)", g=2
                    )
                    nc.scalar.dma_start(
                        dst, xps[n][q][:].rearrange("p (g x) -> p g x", g=2)
                    )
    nc.compile()
    return nc


def _to_bf16(a: np.ndarray):
    import ml_dtypes
    return np.ascontiguousarray(a.astype(ml_dtypes.bfloat16))


def _run(inputs, trace=False):
    from concourse.bass_utils import run_bass_kernel_spmd

    x = np.asarray(inputs["x"], np.float32)
    assert x.shape == (B, C, H, W), x.shape
    xbf = _to_bf16(x)
    tensors, rs1, rs2 = _host_prep(inputs)
    nc = _build_nc(rs1, rs2)
    in_maps = []
    for c in range(NCORES):
        m = {"x": np.ascontiguousarray(xbf[NS * c:NS * (c + 1)])}
        m.update(tensors)
        in_maps.append(m)
    res = run_bass_kernel_spmd(
        nc, in_maps, core_ids=list(range(NCORES)), trace=trace
    )
    out = np.concatenate(
        [np.asarray(res.results[c]["y"]).astype(np.float32) for c in range(NCORES)],
        axis=0,
    )
    return out, res


def kernel(**inputs) -> np.ndarray:
    return _run(inputs)[0]


# revision 23
# speedup vs baseline: 1.0124x; 1.0124x over previous
"""KAN-SE (squeeze-excite with 2-layer KAN MLP) Trainium2 kernel.

Full-input contract: kernel(**inputs) takes the complete (32, 512, 64, 64)
f32 batch plus KAN weights, shards the batch across 8 NeuronCores (4
samples per core, data-parallel, weights replicated), and returns the
full f32 output.

The device pipeline runs entirely in bf16 for the bulk data (the gate is
computed in f32): x is rounded to bf16 on the HOST and the device input/
output tensors are declared bf16, which HALVES HBM traffic in both
directions (16.8 MiB in + 16.8 MiB out per core instead of 33.5+33.5).
The result is upcast to f32 on the host. bf16 rounding of x costs ~0.2%
relative error - an order of magnitude inside the tolerance - and removes
any need for on-device dtype conversion, staging pools, or casting DMAs.

Per-core device program (pure SPMD, no collectives):
  - ALL 16 tiles (4 samples x 4 channel-groups of (128, 4096) bf16) load
    up front on the SP HWDGE ring straight into the resident pool (128
    KiB/partition total - the whole batch stays on-chip); the loads carry
    no waits at all.
  - Per-channel raw sums via ScalarE in-place Copy activations whose
    accum_out produces the sum; sum blocks are emitted one sample ahead
    of the KAN so the in-order ScalarE stream never stalls them behind a
    sigmoid that waits on the PE.
  - Per-sample 2-layer KAN on the sums (f32 math on tiny tensors, bf16
    weights/bases feeding the PE). Mean normalization is folded into the
    layer-1 tables / base weights; activations via Sigmoid only (silu(x)
    = x*sigmoid(x) with a DVE multiply). Layer-1 B-spline bases for all
    4 channel groups are computed by ONE batched Cox-de-Boor chain (~20
    VectorE ops) using stride-0 broadcast APs. The einsums run as paired
    PE matmuls: layer 1 packs the 36 (128x64) blocks pairwise into 18
    (128x128) stationary loads with a 2-column rhs (one DVE add fixes up
    h[o] = ps[o,0] + ps[64+o,1]); layer 2 stacks spline blocks k=2j,2j+1
    across partitions (contraction 64 -> 128), 5 matmuls per out-group.
  - Gate scales applied in-place on the bf16 tiles (VectorE, f32 gate
    scalar per partition), stores on the SP ring FIFO behind the loads.
"""

import numpy as np

# ---- problem constants (hardcoded per contract; do not read spec/reference) ----
B, C, H, W = 32, 512, 64, 64
HIDDEN = 64            # max(16, 512 // 8)
KB = 8                 # GRID_SIZE + SPLINE_ORDER = 5 + 3
NCORES = 8
NS = B // NCORES       # samples per core = 4
NG = C // 128          # channel groups of 128 = 4
HWPIX = H * W          # 4096
NB1 = NG * KB + NG     # layer-1 weight blocks: 32 spline + 4 base = 36
NP1 = NB1 // 2         # 18 paired stationary loads

# Batched layer-1 grid table column layout (widths per Cox-de-Boor level):
# [G0: 4 groups x 12 | A1: 4x10 | C1: 4x10 | A2: 4x9 | C2: 4x9 | A3: 4x8 | C3: 4x8]
_B_OFF = {"G0": 0, (1, "A"): 48, (1, "C"): 88, (2, "A"): 128, (2, "C"): 164,
          (3, "A"): 200, (3, "C"): 232}
_B_W = 264

# layer-2 (per-group) gtab layout: [G0(12) | -g_i k=1(10) | g_{i+2} k=1(10)
#                      | -g_i k=2(9) | g_{i+3} k=2(9) | -g_i k=3(8) | g_{i+4} k=3(8)]
_GT_OFF = {"G0": 0, 1: (12, 22), 2: (32, 41), 3: (50, 58)}
_GT_W = 66


def _grid_tables(grid_row: np.ndarray, pre_scale: float = 1.0):
    """(128, 66) constant table + per-level reciprocals for the per-group
    Cox-de-Boor chain (used for layer 2)."""
    g = np.asarray(grid_row, np.float64) * pre_scale
    assert g.shape == (12,)
    h = g[1] - g[0]
    tab = np.zeros((_GT_W,), np.float64)
    tab[0:12] = g
    rs = {}
    for k in (1, 2, 3):
        w = 11 - k
        aoff, coff = _GT_OFF[k]
        tab[aoff:aoff + w] = -g[:w]          # -g_i,      i = 0..10-k
        tab[coff:coff + w] = g[k + 1:12]     # g_{i+k+1}, i = 0..10-k
        rs[k] = float(np.float32(1.0 / (k * h)))
    full = np.tile(tab.astype(np.float32)[None, :], (128, 1))
    return np.ascontiguousarray(full), rs


def _grid_tables_batched(grid_row: np.ndarray, pre_scale: float):
    """(128, 264) table for the group-batched layer-1 chain, with the
    per-level reciprocals folded in:
      A'_k[i] = -g_i / (k h)   (chain adds s/(k h))
      C'_k[i] = g_{i+k+1} / (k h)  (chain subtracts s/(k h))."""
    g = np.asarray(grid_row, np.float64) * pre_scale
    h = g[1] - g[0]
    tab = np.zeros((_B_W,), np.float64)
    tab[0:48] = np.tile(g, 4)
    rs = {}
    for k in (1, 2, 3):
        w = 11 - k
        r = 1.0 / (k * h)
        rs[k] = float(np.float32(r))
        tab[_B_OFF[(k, "A")]:_B_OFF[(k, "A")] + 4 * w] = np.tile(-g[:w] * r, 4)
        tab[_B_OFF[(k, "C")]:_B_OFF[(k, "C")] + 4 * w] = np.tile(g[k + 1:12] * r, 4)
    full = np.tile(tab.astype(np.float32)[None, :], (128, 1))
    return np.ascontiguousarray(full), rs


def _host_prep(inputs):
    """Rearrange weights into the SBUF layouts the device program uses."""
    import ml_dtypes
    f32 = np.float32
    bf16 = ml_dtypes.bfloat16
    base_w1 = np.asarray(inputs["base_w1"], f32)      # (64, 512)
    spline_w1 = np.asarray(inputs["spline_w1"], f32)  # (64, 512, 8)
    scaler1 = np.asarray(inputs["scaler1"], f32)      # (64, 512)
    base_w2 = np.asarray(inputs["base_w2"], f32)      # (512, 64)
    spline_w2 = np.asarray(inputs["spline_w2"], f32)  # (512, 64, 8)
    scaler2 = np.asarray(inputs["scaler2"], f32)      # (512, 64)

    # Layer-1 blocks in rhs-column order: j<32 spline (g = j//8, k = j%8),
    # j>=32 base path for group g = j-32 (consumes q = s*sigmoid(s/HWPIX) =
    # HWPIX*silu(mean), so the base weights carry the 1/HWPIX).
    sw1 = (spline_w1 * scaler1[:, :, None]).reshape(HIDDEN, NG, 128, KB)
    sw1 = sw1.transpose(2, 1, 3, 0).reshape(128, NG * KB * HIDDEN)
    w1t = base_w1.reshape(HIDDEN, NG, 128).transpose(2, 1, 0).reshape(128, NG * HIDDEN)
    w1t = w1t / float(HWPIX)
    wpack1 = np.concatenate([sw1, w1t], axis=1)       # (128, 36*64)

    # Layer 2: base blocks w2t[p, o] = base_w2[o, p]; spline blocks stacked
    # pairwise across partitions (contraction 64 -> 128).
    w2t = np.ascontiguousarray(base_w2.T)             # (64, 512)
    sw2 = (spline_w2 * scaler2[:, :, None]).transpose(1, 2, 0)  # (64, 8, 512)
    w2pack = np.zeros((128, NG * 4 * 128), f32)
    for og in range(NG):
        for j in range(4):
            blk = slice((og * 4 + j) * 128, (og * 4 + j + 1) * 128)
            w2pack[0:64, blk] = sw2[:, 2 * j, 128 * og:128 * (og + 1)]
            w2pack[64:128, blk] = sw2[:, 2 * j + 1, 128 * og:128 * (og + 1)]

    # Layer 1 evaluates splines on raw per-channel sums: fold the 1/HWPIX
    # mean into the knot tables.
    gtb1, rs1 = _grid_tables_batched(np.asarray(inputs["grid1"], f32)[0],
                                     pre_scale=float(HWPIX))
    gt2, rs2 = _grid_tables(np.asarray(inputs["grid2"], f32)[0])

    tensors = {
        "wpack1": np.ascontiguousarray(wpack1.astype(bf16)),
        "w2pack": np.ascontiguousarray(w2pack.astype(bf16)),
        "w2t": np.ascontiguousarray(w2t.astype(bf16)),
        "gtb1": gtb1,
        "gt2": gt2,
    }
    return tensors, rs1, rs2


def _emit_bsplines(nc, mybir, pool, gt_sb, x_ap, out_ap, p, rs):
    """Per-group cubic B-spline bases (layer 2): x (one value per partition)
    -> out_ap (p, 8). Cox-de-Boor on VectorE."""
    f32 = mybir.dt.float32
    Alu = mybir.AluOpType
    ge = pool.tile([128, 12], f32, tag="ge", bufs=4)
    nc.vector.tensor_scalar(
        out=ge[:p], in0=gt_sb[:p, 0:12], scalar1=x_ap, scalar2=None, op0=Alu.is_le
    )
    bprev = pool.tile([128, 11], f32, tag="b0", bufs=4)
    nc.vector.tensor_tensor(bprev[:p], ge[:p, 0:11], ge[:p, 1:12], Alu.subtract)
    for k in (1, 2, 3):
        w = 11 - k
        aoff, coff = _GT_OFF[k]
        a_t = pool.tile([128, 10], f32, tag="bsA", bufs=4)
        c_t = pool.tile([128, 10], f32, tag="bsC", bufs=4)
        nc.vector.tensor_scalar(
            out=a_t[:p, :w], in0=gt_sb[:p, aoff:aoff + w], scalar1=x_ap,
            scalar2=rs[k], op0=Alu.add, op1=Alu.mult,
        )
        nc.vector.tensor_scalar(
            out=c_t[:p, :w], in0=gt_sb[:p, coff:coff + w], scalar1=x_ap,
            scalar2=rs[k], op0=Alu.subtract, op1=Alu.mult,
        )
        if k < 3:
            bnext = pool.tile([128, 10], f32, tag="bn", bufs=4)
            outp = bnext[:p, :w]
        else:
            outp = out_ap
        nc.vector.tensor_tensor(c_t[:p, :w], c_t[:p, :w], bprev[:p, 1:w + 1], Alu.mult)
        nc.vector.tensor_tensor(outp, a_t[:p, :w], bprev[:p, 0:w], Alu.mult)
        nc.vector.tensor_tensor(outp, outp, c_t[:p, :w], Alu.add)
        if k < 3:
            bprev = bnext


def _emit_bsplines_batched(nc, mybir, pool, spool, gtb_sb, sT, out_ap, rs):
    """Group-batched layer-1 Cox-de-Boor: sums sT (128, 4) -> out_ap
    viewed (128, 4, 8). One VectorE op per step for all 4 groups, using
    stride-0 broadcast APs of sT / (sT * rs_k) along the basis index."""
    f32 = mybir.dt.float32
    Alu = mybir.AluOpType

    # sr[:, 4(k-1):4k] = sT * rs_k  (the s/(k h) broadcast operand)
    sr = spool.tile([128, 12], f32, tag="sr")
    for k in (1, 2, 3):
        nc.vector.tensor_scalar(
            out=sr[:, 4 * (k - 1):4 * k], in0=sT[:], scalar1=rs[k], scalar2=None,
            op0=Alu.mult,
        )

    g3 = lambda ap, w: ap.rearrange("p (g i) -> p g i", g=NG)  # noqa: E731
    ge = pool.tile([128, 48], f32, tag="ge4")
    gev = g3(ge[:], 12)
    nc.vector.tensor_tensor(
        gev, g3(gtb_sb[:, 0:48], 12), sT[:].broadcast_to([128, NG, 12]), Alu.is_le
    )
    bprev = pool.tile([128, 44], f32, tag="b04")
    bpv = g3(bprev[:], 11)
    nc.vector.tensor_tensor(bpv, gev[:, :, 0:11], gev[:, :, 1:12], Alu.subtract)
    for k in (1, 2, 3):
        w = 11 - k
        srb = sr[:, 4 * (k - 1):4 * k].broadcast_to([128, NG, w])
        av = g3(gtb_sb[:, _B_OFF[(k, "A")]:_B_OFF[(k, "A")] + 4 * w], w)
        cv = g3(gtb_sb[:, _B_OFF[(k, "C")]:_B_OFF[(k, "C")] + 4 * w], w)
        a_t = pool.tile([128, 40], f32, tag="bsA4", bufs=2)
        c_t = pool.tile([128, 40], f32, tag="bsC4", bufs=2)
        atv = g3(a_t[:, :4 * w], w)
        ctv = g3(c_t[:, :4 * w], w)
        # A = (s - g_i)/(kh) = s*r + (-g_i*r);  C = (g_{i+k+1} - s)/(kh)
        nc.vector.tensor_tensor(atv, av, srb, Alu.add)
        nc.vector.tensor_tensor(ctv, cv, srb, Alu.subtract)
        if k < 3:
            bnext = pool.tile([128, 40], f32, tag="bn4", bufs=2)
            outp = g3(bnext[:, :4 * w], w)
        else:
            outp = out_ap
        nc.vector.tensor_tensor(ctv, ctv, bpv[:, :, 1:w + 1], Alu.mult)
        nc.vector.tensor_tensor(outp, atv, bpv[:, :, 0:w], Alu.mult)
        nc.vector.tensor_tensor(outp, outp, ctv, Alu.add)
        if k < 3:
            bprev, bpv = bnext, outp


def _build_nc(rs1, rs2):
    import concourse.bacc as bacc
    import concourse.bass as bass  # noqa: F401
    import concourse.mybir as mybir
    from concourse.tile import TileContext

    f32 = mybir.dt.float32
    bf16 = mybir.dt.bfloat16
    Alu = mybir.AluOpType
    Act = mybir.ActivationFunctionType

    # Bacc (not plain Bass): its compile() runs move_matmul_waits_to_ldweights
    # + generate_event_semaphores, which split multi-waits down to the 1-wait-
    # per-instruction TRN2 ISA limit that walrus enforces.
    nc = bacc.Bacc("TRN2", target_bir_lowering=False)
    x_d = nc.declare_dram_parameter("x", [NS, C, H, W], bf16, isOutput=False)
    wp1_d = nc.declare_dram_parameter("wpack1", [128, NB1 * HIDDEN], bf16, isOutput=False)
    wp2_d = nc.declare_dram_parameter("w2pack", [128, NG * 4 * 128], bf16, isOutput=False)
    w2t_d = nc.declare_dram_parameter("w2t", [HIDDEN, C], bf16, isOutput=False)
    gtb1_d = nc.declare_dram_parameter("gtb1", [128, _B_W], f32, isOutput=False)
    gt2_d = nc.declare_dram_parameter("gt2", [128, _GT_W], f32, isOutput=False)
    y_d = nc.declare_dram_parameter("y", [NS, C, H, W], bf16, isOutput=True)

    with TileContext(nc) as tc:
        with (
            tc.tile_pool(name="consts", bufs=1) as cpool,
            tc.tile_pool(name="xdata", bufs=NS * NG // 2) as xpool,
            tc.tile_pool(name="small", bufs=3) as spool,
            tc.tile_pool(name="bspl", bufs=1) as bpool,
            tc.tile_pool(name="psum", bufs=2, space="PSUM") as ppool,
        ):
            # Constants on the ACT HWDGE ring: off the x-load (SP) ring.
            # Grid tables first: the first sample's B-splines need them early.
            gtb1_sb = cpool.tile([128, _B_W], f32)
            nc.scalar.dma_start(gtb1_sb[:], gtb1_d[:, :])
            gt2_sb = cpool.tile([128, _GT_W], f32)
            nc.scalar.dma_start(gt2_sb[:], gt2_d[:, :])
            wp1_sb = cpool.tile([128, NB1 * HIDDEN], bf16)
            nc.scalar.dma_start(wp1_sb[:], wp1_d[:, :])
            wp2_sb = cpool.tile([128, NG * 4 * 128], bf16)
            nc.scalar.dma_start(wp2_sb[:], wp2_d[:, :])
            w2t_sb = cpool.tile([HIDDEN, C], bf16)
            nc.scalar.dma_start(w2t_sb[:], w2t_d[:, :])

            # Pre-touch every const tile on VectorE: the DMA-completion wait
            # lands on these throwaway copies, so later DVE consumers (notably
            # TensorScalarPtr ops, whose ISA format has a single wait slot)
            # never need a DMA wait of their own.
            touch = cpool.tile([128, 8], f32)
            for i, ct in enumerate((wp1_sb, wp2_sb, gtb1_sb, gt2_sb)):
                nc.vector.tensor_copy(touch[:, i:i + 1], ct[:, 0:1])
            nc.vector.tensor_copy(touch[:HIDDEN, 4:5], w2t_sb[:, 0:1])
            # Same for TensorE: the LDWEIGHTS sub-instruction also has a single
            # wait slot, so absorb each weight tile's DMA wait into a throwaway
            # 1-column matmul before the real accumulation chains.
            pt_ps = ppool.tile([1, 4], f32, tag="pt")
            for i, ct in enumerate((wp1_sb, wp2_sb)):
                nc.tensor.matmul(pt_ps[0:1, i:i + 1], ct[:, 0:1], ct[:, 0:1],
                                 start=True, stop=True)
            nc.tensor.matmul(pt_ps[0:1, 2:3], w2t_sb[:HIDDEN, 0:1],
                             w2t_sb[:HIDDEN, 0:1], start=True, stop=True)

            # ---- ALL x loads up front on the SP HWDGE ring (bf16, no
            # staging, no waits: the whole batch is SBUF-resident).
            # 2 MiB pair-DMAs (channel groups 2q, 2q+1 per transfer) for
            # better per-DMA efficiency; group tiles are views of the pair.
            xps = [[None] * (NG // 2) for _ in range(NS)]
            xts = [[None] * NG for _ in range(NS)]
            for n in range(NS):
                for q in range(NG // 2):
                    xp = xpool.tile([128, 2 * HWPIX], bf16, tag="xt")
                    src = x_d[n, 256 * q:256 * (q + 1)].rearrange(
                        "(g p) h w -> p g (h w)", g=2
                    )
                    nc.sync.dma_start(
                        xp[:].rearrange("p (g x) -> p g x", g=2), src
                    )
                    xps[n][q] = xp
                    xts[n][2 * q] = xp[:, 0:HWPIX]
                    xts[n][2 * q + 1] = xp[:, HWPIX:2 * HWPIX]

            sTs = [None] * NS

            def emit_sums(n):
                # per-channel raw sums, split across engines: groups 0,1 via
                # ScalarE in-place Copy+accum_out; groups 2,3 via VectorE
                # tensor_tensor_reduce (in-place max(x,x) identity whose
                # accumulator is the free-dim sum, at 2x bf16 TT rate)
                sT_n = spool.tile([128, NG], f32, tag="sT", bufs=4)
                sTs[n] = sT_n
                for g in range(NG):
                    nc.scalar.activation(
                        xts[n][g], xts[n][g], Act.Copy,
                        accum_out=sT_n[:, g:g + 1],
                    )

            # Sum blocks one sample AHEAD of the KAN: on the in-order ScalarE
            # stream sample n+1's sums run before sample n's sigmoid chain
            # (which waits on the PE), keeping the gate cadence at the
            # engine-throughput limit rather than the serial-chain limit.
            emit_sums(0)
            for n in range(NS):
                if n + 1 < NS:
                    emit_sums(n + 1)
                sT = sTs[n]

                # ---- KAN layer 1: 18 paired matmuls; bfq holds the 36
                # rhs betas (32 spline via batched chain + 4 base) ----
                bfq = spool.tile([128, NB1], bf16, tag="bfq")
                bff = spool.tile([128, NG * KB], f32, tag="bff")
                sg = spool.tile([128, NG], f32, tag="sg")
                nc.scalar.activation(sg[:], sT[:], Act.Sigmoid, scale=1.0 / HWPIX)
                # base-path betas: q = s * sigmoid(s/HWPIX)  (cols 32..35)
                nc.vector.tensor_tensor(bfq[:, NG * KB:NB1], sg[:], sT[:], Alu.mult)
                _emit_bsplines_batched(
                    nc, mybir, bpool, spool, gtb1_sb, sT,
                    bff[:].rearrange("p (g i) -> p g i", g=NG), rs1,
                )
                nc.vector.tensor_copy(bfq[:, 0:NG * KB], bff[:])
                psA = ppool.tile([128, 2], f32, tag="ps1")
                for j in range(NP1):
                    nc.tensor.matmul(
                        psA[:, 0:2], wp1_sb[:, 128 * j:128 * (j + 1)],
                        bfq[:, 2 * j:2 * j + 2],
                        start=(j == 0), stop=(j == NP1 - 1),
                    )
                hv = spool.tile([HIDDEN, 1], f32, tag="hv")
                # DVE may read only one PSUM operand per instruction
                nc.vector.tensor_copy(hv[:], psA[HIDDEN:128, 1:2])
                nc.vector.tensor_tensor(hv[:], hv[:], psA[0:HIDDEN, 0:1], Alu.add)

                # ---- inter-layer SiLU (t1 = h*sigmoid(h)), layer 2 ----
                sg1 = spool.tile([HIDDEN, 1], f32, tag="sg1")
                nc.scalar.activation(sg1[:], hv[:], Act.Sigmoid)
                t1 = spool.tile([HIDDEN, 1], f32, tag="t1")
                nc.vector.tensor_tensor(t1[:], sg1[:], hv[:], Alu.mult)
                sg2 = spool.tile([HIDDEN, 1], f32, tag="sg2")
                nc.scalar.activation(sg2[:], t1[:], Act.Sigmoid)
                silu2 = spool.tile([HIDDEN, 1], bf16, tag="silu2")
                nc.vector.tensor_tensor(silu2[:], sg2[:], t1[:], Alu.mult)
                b2f = spool.tile([HIDDEN, KB], f32, tag="b2f")
                _emit_bsplines(nc, mybir, bpool, gt2_sb, t1[:, 0:1], b2f[:], HIDDEN, rs2)

                # stacked rhs: stk[0:64, j] = b2f[:, 2j], stk[64:128, j] = b2f[:, 2j+1]
                stk = spool.tile([128, 4], bf16, tag="stk")
                nc.vector.tensor_copy(stk[0:HIDDEN, 0:4], b2f[:, 0:KB:2])
                nc.vector.tensor_copy(stk[HIDDEN:128, 0:4], b2f[:, 1:KB:2])

                ps2 = ppool.tile([128, NG], f32, tag="ps2")
                for og in range(NG):
                    nc.tensor.matmul(
                        ps2[:, og:og + 1], w2t_sb[:, 128 * og:128 * (og + 1)],
                        silu2[:, 0:1], start=True, stop=False,
                    )
                    for j in range(4):
                        blk = slice((og * 4 + j) * 128, (og * 4 + j + 1) * 128)
                        nc.tensor.matmul(
                            ps2[:, og:og + 1], wp2_sb[:, blk], stk[:, j:j + 1],
                            start=False, stop=(j == 3),
                        )

                gate = spool.tile([128, NG], f32, tag="gate")
                nc.scalar.activation(gate[:], ps2[:], Act.Sigmoid)

                # ---- scale resident bf16 tiles in place (VectorE), then
                # store 2 MiB pairs on the SP ring: its load DMAs were all
                # issued long ago, so stores queue FIFO in gate order ----
                for g in range(NG):
                    nc.vector.tensor_scalar(
                        out=xts[n][g], in0=xts[n][g],
                        scalar1=gate[:, g:g + 1], scalar2=None, op0=Alu.mult,
                    )
                for q in range(NG // 2):
                    dst = y_d[n, 256 * q:256 * (q + 1)].rearrange(
                        "(g p) h w -> p g (h w)", g=2
                    )
                    nc.sync.dma_start(
                        dst, xps[n][q][:].rearrange("p (g x) -> p g x", g=2)
                    )
    nc.compile()
    return nc


def _to_bf16(a: np.ndarray):
    import ml_dtypes
    return np.ascontiguousarray(a.astype(ml_dtypes.bfloat16))


def _run(inputs, trace=False):
    from concourse.bass_utils import run_bass_kernel_spmd

    x = np.asarray(inputs["x"], np.float32)
    assert x.shape == (B, C, H, W), x.shape
    xbf = _to_bf16(x)
    tensors, rs1, rs2 = _host_prep(inputs)
    nc = _build_nc(rs1, rs2)
    in_maps = []
    for c in range(NCORES):
        m = {"x": np.ascontiguousarray(xbf[NS * c:NS * (c + 1)])}
        m.update(tensors)
        in_maps.append(m)
    res = run_bass_kernel_spmd(
        nc, in_maps, core_ids=list(range(NCORES)), trace=trace
    )
    out = np.concatenate(
        [np.asarray(res.results[c]["y"]).astype(np.float32) for c in range(NCORES)],
        axis=0,
    )
    return out, res


def kernel(**inputs) -> np.ndarray:
    return _run(inputs)[0]
